# revision 1
# baseline (speedup 1.0000x reference)
"""Trainium2 Bass kernel for linear multi-head attention (elu+1 feature map).

Math (per batch n):
  q = x_q @ Wq.T ; k = x_k @ Wk.T ; v = (x_v @ Wv.T) / L
  Q = elu(q)+1 ; K = elu(k)+1
  KV[h] = K_h.T @ v_h              (D x D per head)
  Ksum  = sum_s K[s, :]            (E)
  S[l,h] = Q_h[l] . Ksum_h ;  W = L / (S + eps)
  msg[l, h*D+dv] = (Q_h[l] @ KV[h])[dv] * W[l,h]
  out = msg @ Wm.T

Sharding: B*L = 16384 rows split into 8 chunks of 2048 (each core gets half
of one batch's sequence). Only cross-core dependency: the KV/Ksum reduction
between the two cores sharing a batch -> pairwise AllReduce of 67.6KB.

Dtypes: projections/merge in float32r (1 cyc/row, fp32 storage). Q/S/msg
matmuls in bf16 (required for tile_position packing; S positive-sum
averaging keeps Ksum error negligible). KV runs as an fp32r full
cross-product K^T V with diagonal D x D blocks extracted afterwards.
"""

import numpy as np

B = 4
L = 4096
E = 512
H = 16
D = 32
P = 128
KT = E // P
NCORES = 8
R = (B * L) // NCORES
ST = R // P
NCHUNK = 4
CH = R // NCHUNK
EPS = 1e-6
CC = H * D * D + E

_CACHE = {}
LAST_EXEC_NS = None
LAST_RESULTS = None


def _build():
    import concourse.bass as bass
    import concourse.mybir as mybir
    import concourse.tile as tile
    from concourse import bacc

    f32 = mybir.dt.float32
    f32r = mybir.dt.float32r
    bf16 = mybir.dt.bfloat16
    AFT = mybir.ActivationFunctionType
    OP = mybir.AluOpType

    nc = bacc.Bacc("TRN2", target_bir_lowering=False, debug=False,
                   num_devices=NCORES)

    xq_d = nc.dram_tensor("xq", [E, R], f32r, kind="ExternalInput").ap()
    xk_d = nc.dram_tensor("xk", [E, R], f32r, kind="ExternalInput").ap()
    xv_d = nc.dram_tensor("xv", [E, R], f32r, kind="ExternalInput").ap()
    wq_d = nc.dram_tensor("wq", [E, E], f32r, kind="ExternalInput").ap()
    wk_d = nc.dram_tensor("wk", [E, E], f32r, kind="ExternalInput").ap()
    wv_d = nc.dram_tensor("wv", [E, E], f32r, kind="ExternalInput").ap()
    wm_d = nc.dram_tensor("wm", [E, E], f32r, kind="ExternalInput").ap()
    out_d = nc.dram_tensor("out", [R, E], f32, kind="ExternalOutput").ap()

    RG = [[0, 1], [2, 3], [4, 5], [6, 7]]

    with tile.TileContext(nc) as tc:

        def elu1(tpool, ps_in, out_ap, tag):
            """out = elu(x)+1 = Exp(-Relu(-x)) + max(x,0); 2 ACT + 1 DVE."""
            n = ps_in.shape[-1]
            tA = tpool.tile([P, n], f32, name=f"tA{tag}", tag=f"tA{tag}")
            tB = tpool.tile([P, n], f32, name=f"tB{tag}", tag=f"tB{tag}")
            nc.scalar.activation(tA[:], ps_in, AFT.Relu, scale=-1.0)
            nc.scalar.activation(tB[:], tA[:], AFT.Exp, scale=-1.0)
            nc.vector.scalar_tensor_tensor(
                out_ap, ps_in, 0.0, tB[:], OP.max, OP.add)

        with tc.tile_pool(name="const", bufs=1) as const, \
             tc.tile_pool(name="xq_pool", bufs=1) as xq_pool, \
             tc.tile_pool(name="qt_pool", bufs=1) as qt_pool, \
             tc.tile_pool(name="dram", bufs=1, space="DRAM") as dram:

            # ---- weights (k/v first: phase A needs them immediately) ----
            wq_sb = const.tile([P, KT, E], f32r)
            wk_sb = const.tile([P, KT, E], f32r)
            wv_sb = const.tile([P, KT, E], f32r)
            wm_sb = const.tile([P, KT, E], f32r)
            ones_f32 = const.tile([P, 1], f32)
            nc.vector.memset(ones_f32[:], 1.0)
            ones_sb = const.tile([P, 1], f32r)
            nc.vector.tensor_copy(ones_sb[:], ones_f32[:])

            cc_in = dram.tile([CC], f32)
            cc_out = dram.tile([CC], f32)

            # =================== Phase A: k/v proj + KV/Ksum ===============
            with tc.tile_pool(name="xkv_pool", bufs=1) as xkv_pool, \
                 tc.tile_pool(name="workA", bufs=3) as workA, \
                 tc.tile_pool(name="psA", bufs=3, space="PSUM") as psA, \
                 tc.tile_pool(name="kvp", bufs=1, space="PSUM") as kvp:

                xk_sb = xkv_pool.tile([P, KT, R], f32r)
                xv_sb = xkv_pool.tile([P, KT, R], f32r)
                wv_r = wv_d.rearrange("(ko ki) n -> ki ko n", ki=P)
                wk_r = wk_d.rearrange("(ko ki) n -> ki ko n", ki=P)
                # interleave (weight ko, x ko) pairs so s-tile 0 unblocks
                # after ~2MB instead of after all weights
                for ko in range(KT):
                    nc.sync.dma_start(wv_sb[:, ko], wv_r[:, ko])
                    nc.sync.dma_start(
                        xv_sb[:, ko, 0:CH], xv_d[ko * P:(ko + 1) * P, 0:CH])
                    nc.sync.dma_start(wk_sb[:, ko], wk_r[:, ko])
                    nc.sync.dma_start(
                        xk_sb[:, ko, 0:CH], xk_d[ko * P:(ko + 1) * P, 0:CH])
                for ko in range(KT):
                    nc.sync.dma_start(
                        xk_sb[:, ko, CH:R], xk_d[ko * P:(ko + 1) * P, CH:R])
                    nc.sync.dma_start(
                        xv_sb[:, ko, CH:R], xv_d[ko * P:(ko + 1) * P, CH:R])

                # q-side operands go down the Activation HWDGE queue so the
                # two HW queues stream in parallel
                for w_sb, w_d in ((wq_sb, wq_d), (wm_sb, wm_d)):
                    nc.scalar.dma_start(
                        w_sb[:], w_d.rearrange("(ko ki) n -> ki ko n", ki=P))
                xq_sb = xq_pool.tile([P, KT, R], f32r)
                for ko in range(KT):
                    nc.scalar.dma_start(
                        xq_sb[:, ko, :], xq_d[ko * P:(ko + 1) * P, :])

                kvf_ps = [kvp.tile([P, E], f32, name=f"kvf{g}")
                          for g in range(4)]
                ksum_ps = kvp.tile([1, E], f32)

                # software pipeline: KV(si-1) emitted between projections of
                # si so the PE never waits for the elu chain
                kv_prev = None
                kv_stage = None
                for si in range(ST + 1):
                    if si < ST:
                        sl = slice(si * P, (si + 1) * P)
                        v_ps = psA.tile([P, E], f32, name="v_ps", tag="proj")
                        for ko in range(KT):
                            nc.tensor.matmul(
                                v_ps[:], xv_sb[:, ko, sl], wv_sb[:, ko, :],
                                start=(ko == 0), stop=(ko == KT - 1))
                        v_sb = workA.tile([P, E], f32r, name="v_sb")
                        nc.scalar.copy(v_sb[:], v_ps[:])

                        k_ps = psA.tile([P, E], f32, name="k_ps", tag="proj")
                        for ko in range(KT):
                            nc.tensor.matmul(
                                k_ps[:], xk_sb[:, ko, sl], wk_sb[:, ko, :],
                                start=(ko == 0), stop=(ko == KT - 1))
                        k_sb = workA.tile([P, E], f32r, name="k_sb")
                        elu1(workA, k_ps[:], k_sb[:], "k")
                        kv_stage = (k_sb, v_sb)
                    if si >= 1:
                        pk, pv = kv_prev
                        for g in range(4):
                            nc.tensor.matmul(
                                kvf_ps[g][:], pk[:, g * P:(g + 1) * P], pv[:],
                                start=(si == 1), stop=(si == ST))
                        nc.tensor.matmul(
                            ksum_ps[:, :], ones_sb[:, 0:1], pk[:],
                            start=(si == 1), stop=(si == ST))
                    kv_prev = kv_stage

                # extract the 16 diagonal DxD blocks into packed layout
                kv_sb = workA.tile([P, P], f32, name="kv_sb")
                for g in range(4):
                    for j in range(4):
                        h = 4 * g + j
                        nc.vector.tensor_copy(
                            kv_sb[32 * j:32 * (j + 1), 32 * g:32 * (g + 1)],
                            kvf_ps[g][32 * j:32 * (j + 1),
                                      32 * h:32 * (h + 1)])
                ksum_sb = workA.tile([1, E], f32, name="ksum_sb")
                nc.vector.tensor_copy(ksum_sb[:], ksum_ps[:])
                nc.sync.dma_start(
                    cc_in[0:P * P].rearrange("(p f) -> p f", p=P), kv_sb[:])
                nc.sync.dma_start(
                    cc_in[P * P:CC].rearrange("(p f) -> p f", p=1), ksum_sb[:])

            # =================== pairwise AllReduce =========================
            nc.gpsimd.collective_compute(
                "AllReduce", mybir.AluOpType.add, replica_groups=RG,
                ins=[cc_in[:].opt()], outs=[cc_out[:].opt()])

            kv_red = const.tile([P, P], f32)
            nc.sync.dma_start(
                kv_red[:], cc_out[0:P * P].rearrange("(p f) -> p f", p=P))
            kv_bf = const.tile([P, P], bf16)
            nc.vector.tensor_copy(kv_bf[:], kv_red[:])
            # block-diagonal Ksum built arithmetically: bd = bcast(ks)*mask
            # mask[p, g, c] = (c == 4g + p//32); constant baked into the NEFF
            mask_np = np.zeros((P, KT, H), np.float32)
            for g in range(4):
                for j in range(4):
                    mask_np[32 * j:32 * (j + 1), g, 4 * g + j] = 1.0
            mask_d = nc.inline_tensor(mask_np, name="bd_mask")
            mask_sb = const.tile([P, KT, H], f32)
            nc.sync.dma_start(mask_sb[:], mask_d.ap())
            ks_f32 = const.tile([P, KT], f32)
            nc.sync.dma_start(
                ks_f32[:], cc_out[P * P:CC].rearrange("(g p) -> p g", p=P))
            bd_bf = const.tile([P, KT, H], bf16)
            for g in range(4):
                nc.vector.tensor_tensor(
                    bd_bf[:, g, :], mask_sb[:, g, :],
                    ks_f32[:, g, None].to_broadcast((P, H)), OP.mult)

            # =================== Phase B ====================================
            with tc.tile_pool(name="workB", bufs=3) as workB, \
                 tc.tile_pool(name="psB", bufs=2, space="PSUM") as psB:

                # q projection + elu for ALL chunks first (PE stays busy
                # while the collective completes); Q is stored bf16.
                qt_sb = qt_pool.tile([P, KT, R], bf16)
                for c in range(NCHUNK):
                    cs = slice(c * CH, (c + 1) * CH)
                    for no in range(KT):
                        q_ps = psB.tile([P, CH], f32, name="q_ps")
                        for ko in range(KT):
                            nc.tensor.matmul(
                                q_ps[:], wq_sb[:, ko, no * P:(no + 1) * P],
                                xq_sb[:, ko, cs],
                                start=(ko == 0), stop=(ko == KT - 1))
                        elu1(workB, q_ps[:], qt_sb[:, no, cs], "q")

                # S for all chunks early, so the W chains (DVE) overlap with
                # PE work instead of serializing each chunk
                w16s = []
                for c in range(NCHUNK):
                    cs = slice(c * CH, (c + 1) * CH)
                    s_ps = psB.tile([H, CH], f32, name="s_ps")
                    for g in range(4):
                        nc.tensor.matmul(
                            s_ps[:], bd_bf[:, g, :], qt_sb[:, g, cs],
                            start=(g == 0), stop=(g == 3))
                    # W = L / (S + eps) = 1 / (S/L + eps/L)
                    w_t = workB.tile([H, CH], f32, name="w_t", tag="w_t")
                    nc.vector.tensor_scalar(
                        w_t[:], s_ps[:], 1.0 / L, EPS / L, OP.mult, OP.add)
                    w16 = workB.tile([H, CH], f32, name=f"w16_{c}",
                                     tag=f"w16_{c}")
                    nc.vector.reciprocal_approx_fast(w16[:], w_t[:])
                    w16s.append(w16)

                # all W broadcasts early (SWDGE queue, off critical path)
                wbs = {}
                for c in range(NCHUNK):
                    for g in range(4):
                        wb_sb = workB.tile([P, CH], f32, name=f"wb{c}_{g}",
                                           tag=f"wb{c}_{g}", bufs=1)
                        w_ap = w16s[c][4 * g:4 * (g + 1), :]
                        w_b = bass.AP(
                            tensor=w_ap.tensor, offset=w_ap.offset,
                            ap=[list(w_ap.ap[0]), [0, 32], list(w_ap.ap[1])])
                        nc.gpsimd.dma_start(out=wb_sb[:], in_=w_b)
                        wbs[(c, g)] = wb_sb

                # pass 1: all message matmuls + scaling
                msgs = []
                for c in range(NCHUNK):
                    cs = slice(c * CH, (c + 1) * CH)
                    msg_sb = workB.tile([P, KT, CH], f32r, name=f"msg{c}",
                                        tag=f"msg{c}", bufs=1)
                    for g in range(4):
                        m_ps = psB.tile([P, CH], f32, name="m_ps")
                        for j in range(4):
                            sl32 = slice(32 * j, 32 * (j + 1))
                            nc.tensor.matmul(
                                m_ps[sl32, :],
                                kv_bf[sl32, 32 * g:32 * (g + 1)],
                                qt_sb[sl32, g, cs],
                                start=True, stop=True,
                                tile_position=(32 * j, 32 * j))
                        nc.vector.tensor_tensor(
                            msg_sb[:, g, :], m_ps[:], wbs[(c, g)][:], OP.mult)
                    msgs.append(msg_sb)

                # pass 2: merge projections
                for c in range(NCHUNK):
                    for lt in range(CH // P):
                        o_ps = psB.tile([P, E], f32, name="o_ps")
                        for g in range(KT):
                            nc.tensor.matmul(
                                o_ps[:],
                                msgs[c][:, g, lt * P:(lt + 1) * P],
                                wm_sb[:, g, :],
                                start=(g == 0), stop=(g == KT - 1))
                        o_sb = workB.tile([P, E], f32, name="o_sb")
                        nc.scalar.copy(o_sb[:], o_ps[:])
                        nc.scalar.dma_start(
                            out_d[c * CH + lt * P:c * CH + (lt + 1) * P, :],
                            o_sb[:])

    nc.compile()
    return nc


def _get_nc():
    if "nc" not in _CACHE:
        _CACHE["nc"] = _build()
    return _CACHE["nc"]


def kernel(query, key, value, Wq, Wk, Wv, Wm):
    global LAST_EXEC_NS, LAST_RESULTS
    import os
    from concourse.bass_utils import run_bass_kernel_spmd

    query = np.asarray(query, dtype=np.float32)
    key = np.asarray(key, dtype=np.float32)
    value = np.asarray(value, dtype=np.float32)
    wq_t = np.ascontiguousarray(np.asarray(Wq, np.float32).T)
    wk_t = np.ascontiguousarray(np.asarray(Wk, np.float32).T)
    wv_t = np.ascontiguousarray(np.asarray(Wv, np.float32).T / L)
    wm_t = np.ascontiguousarray(np.asarray(Wm, np.float32).T)

    in_maps = []
    for c in range(NCORES):
        b, half = c // 2, c % 2
        rs = slice(half * R, (half + 1) * R)
        in_maps.append({
            "xq": np.ascontiguousarray(query[b, rs, :].T),
            "xk": np.ascontiguousarray(key[b, rs, :].T),
            "xv": np.ascontiguousarray(value[b, rs, :].T),
            "wq": wq_t, "wk": wk_t, "wv": wv_t, "wm": wm_t,
        })

    nc = _get_nc()
    trace = bool(int(os.environ.get("KERNEL_TRACE", "0")))
    res = run_bass_kernel_spmd(nc, in_maps, core_ids=list(range(NCORES)),
                               trace=trace)
    LAST_EXEC_NS = res.exec_time_ns
    LAST_RESULTS = res

    out = np.empty((B, L, E), dtype=np.float32)
    for c in range(NCORES):
        b, half = c // 2, c % 2
        out[b, half * R:(half + 1) * R, :] = res.results[c]["out"]
    return out



# revision 7
# speedup vs baseline: 1.5319x; 1.5319x over previous
"""Trainium2 Bass kernel for linear multi-head attention (elu+1 feature map).

Math (per batch n):
  q = x_q @ Wq.T ; k = x_k @ Wk.T ; v = (x_v @ Wv.T) / L
  Q = elu(q)+1 ; K = elu(k)+1
  KV[h] = K_h.T @ v_h              (D x D per head)
  Ksum  = sum_s K[s, :]            (E)
  S[l,h] = Q_h[l] . Ksum_h ;  W = L / (S + eps)
  msg[l, h*D+dv] = (Q_h[l] @ KV[h])[dv] * W[l,h]
  out = msg @ Wm.T

Sharding: B*L = 16384 rows split into 8 chunks of 2048 (each core gets half
of one batch's sequence). Only cross-core dependency: the KV/Ksum reduction
between the two cores sharing a batch -> pairwise AllReduce (bf16, 33.8KB).

v2 structure (vs. the 219us baseline):
  - all input DMA on the sync (SP) HWDGE queue, phase-A tensors first and
    interleaved at 512-row granularity so the k/v pipeline starts early;
    q-side tensors queued behind them.  No DMA issue on the ACT engine.
  - KV cross-product is group-local (K_g^T @ V_g, N=128) instead of the
    full E x E product.
  - S is computed directly in replicated layout ([128, CH] per group) by
    matmul with a block-masked Ksum operand, so the 1/(S+eps) weights line
    up with the message layout without any partition-broadcast DMA.
  - merge matmuls interleaved one chunk behind the message matmuls.
"""

import numpy as np

B = 4
L = 4096
E = 512
H = 16
D = 32
P = 128
KT = E // P
NCORES = 8
R = (B * L) // NCORES
ST = R // P
NCHUNK = 4
CH = R // NCHUNK
EPS = 1e-6
CC = P * P + E

_CACHE = {}
LAST_EXEC_NS = None
LAST_RESULTS = None


def _build():
    import concourse.bass as bass
    import concourse.mybir as mybir
    import concourse.tile as tile
    from concourse import bacc

    f32 = mybir.dt.float32
    f32r = mybir.dt.float32r
    bf16 = mybir.dt.bfloat16
    AFT = mybir.ActivationFunctionType
    OP = mybir.AluOpType

    nc = bacc.Bacc("TRN2", target_bir_lowering=False, debug=False,
                   num_devices=NCORES)

    xq_d = nc.dram_tensor("xq", [E, R], f32r, kind="ExternalInput").ap()
    xk_d = nc.dram_tensor("xk", [E, R], f32r, kind="ExternalInput").ap()
    xv_d = nc.dram_tensor("xv", [E, R], f32r, kind="ExternalInput").ap()
    wq_d = nc.dram_tensor("wq", [E, E], f32r, kind="ExternalInput").ap()
    wk_d = nc.dram_tensor("wk", [E, E], f32r, kind="ExternalInput").ap()
    wv_d = nc.dram_tensor("wv", [E, E], f32r, kind="ExternalInput").ap()
    wm_d = nc.dram_tensor("wm", [E, E], f32r, kind="ExternalInput").ap()
    out_d = nc.dram_tensor("out", [R, E], f32, kind="ExternalOutput").ap()

    RG = [[0, 1], [2, 3], [4, 5], [6, 7]]

    with tile.TileContext(nc) as tc:

        def elu1(tpool, ps_in, out_ap, tag):
            """out = elu(x)+1 = Exp(-Relu(-x)) + max(x,0); 2 ACT + 1 DVE."""
            n = ps_in.shape[-1]
            tA = tpool.tile([P, n], f32, name=f"tA{tag}", tag=f"tA{tag}")
            tB = tpool.tile([P, n], f32, name=f"tB{tag}", tag=f"tB{tag}")
            nc.scalar.activation(tA[:], ps_in, AFT.Relu, scale=-1.0)
            nc.scalar.activation(tB[:], tA[:], AFT.Exp, scale=-1.0)
            nc.vector.scalar_tensor_tensor(
                out_ap, ps_in, 0.0, tB[:], OP.max, OP.add)

        with tc.tile_pool(name="const", bufs=1) as const, \
             tc.tile_pool(name="xq_pool", bufs=1) as xq_pool, \
             tc.tile_pool(name="qt_pool", bufs=1) as qt_pool, \
             tc.tile_pool(name="dram", bufs=1, space="DRAM") as dram:

            wq_sb = const.tile([P, KT, E], f32r)
            wk_sb = const.tile([P, KT, E], f32r)
            wv_sb = const.tile([P, KT, E], f32r)
            wm_sb = const.tile([P, KT, E], f32r)
            ones_f32 = const.tile([P, 1], f32)
            nc.vector.memset(ones_f32[:], 1.0)
            ones_sb = const.tile([P, 1], bf16)
            nc.vector.tensor_copy(ones_sb[:], ones_f32[:])

            # block-identity mask: maskI[k, p] = (k//32 == p//32)
            maskI_np = np.zeros((P, P), np.float32)
            for j in range(4):
                maskI_np[32 * j:32 * (j + 1), 32 * j:32 * (j + 1)] = 1.0
            maskI_d = nc.inline_tensor(maskI_np, name="blk_ident")
            maskI_sb = const.tile([P, P], f32)
            nc.gpsimd.dma_start(maskI_sb[:], maskI_d.ap())

            cc_in = dram.tile([CC], f32)
            cc_out = dram.tile([CC], f32)

            wv_r = wv_d.rearrange("(ko ki) n -> ki ko n", ki=P)
            wk_r = wk_d.rearrange("(ko ki) n -> ki ko n", ki=P)
            wq_r = wq_d.rearrange("(ko ki) n -> ki ko n", ki=P)
            wm_r = wm_d.rearrange("(ko ki) n -> ki ko n", ki=P)

            # =================== Phase A: k/v proj + KV/Ksum ===============
            with tc.tile_pool(name="xkv_pool", bufs=1) as xkv_pool, \
                 tc.tile_pool(name="workA", bufs=3) as workA, \
                 tc.tile_pool(name="psA", bufs=3, space="PSUM") as psA, \
                 tc.tile_pool(name="kvp", bufs=1, space="PSUM") as kvp:

                xk_sb = xkv_pool.tile([P, KT, R], f32r)
                xv_sb = xkv_pool.tile([P, KT, R], f32r)
                xq_sb = xq_pool.tile([P, KT, R], f32r)

                # ---- DMA order on the sync queue (SP engine): weights +
                # first 512 rows of k/v, then the rest of k/v in 512-row
                # stripes, then wq, xq (chunk-major), wm.
                for ko in range(KT):
                    nc.sync.dma_start(wv_sb[:, ko], wv_r[:, ko])
                    nc.sync.dma_start(
                        xv_sb[:, ko, 0:CH], xv_d[ko * P:(ko + 1) * P, 0:CH])
                    nc.sync.dma_start(wk_sb[:, ko], wk_r[:, ko])
                    nc.sync.dma_start(
                        xk_sb[:, ko, 0:CH], xk_d[ko * P:(ko + 1) * P, 0:CH])
                for sc in range(1, NCHUNK):
                    cs = slice(sc * CH, (sc + 1) * CH)
                    for ko in range(KT):
                        nc.sync.dma_start(
                            xv_sb[:, ko, cs], xv_d[ko * P:(ko + 1) * P, cs])
                        nc.sync.dma_start(
                            xk_sb[:, ko, cs], xk_d[ko * P:(ko + 1) * P, cs])
                for ko in range(KT):
                    nc.sync.dma_start(wq_sb[:, ko], wq_r[:, ko])
                for c in range(NCHUNK):
                    cs = slice(c * CH, (c + 1) * CH)
                    for ko in range(KT):
                        nc.sync.dma_start(
                            xq_sb[:, ko, cs], xq_d[ko * P:(ko + 1) * P, cs])
                for ko in range(KT):
                    nc.sync.dma_start(wm_sb[:, ko], wm_r[:, ko])

                # group-local KV accumulator: kvacc[:, g, :] = K_g^T @ V_g
                # (4 x [128,128] side by side = exactly one PSUM bank)
                kvacc = kvp.tile([P, KT, P], f32)
                ksum_ps = kvp.tile([1, E], f32)

                # software pipeline: KV(si-1) emitted between projections of
                # si so the PE never waits for the elu chain
                kv_prev = None
                kv_stage = None
                for si in range(ST + 1):
                    if si < ST:
                        sl = slice(si * P, (si + 1) * P)
                        v_ps = psA.tile([P, E], f32, name="v_ps", tag="proj")
                        for ko in range(KT):
                            nc.tensor.matmul(
                                v_ps[:], xv_sb[:, ko, sl], wv_sb[:, ko, :],
                                start=(ko == 0), stop=(ko == KT - 1))
                        v_sb = workA.tile([P, E], bf16, name="v_sb")
                        nc.scalar.copy(v_sb[:], v_ps[:])

                        k_ps = psA.tile([P, E], f32, name="k_ps", tag="proj")
                        for ko in range(KT):
                            nc.tensor.matmul(
                                k_ps[:], xk_sb[:, ko, sl], wk_sb[:, ko, :],
                                start=(ko == 0), stop=(ko == KT - 1))
                        k_sb = workA.tile([P, E], bf16, name="k_sb")
                        elu1(workA, k_ps[:], k_sb[:], "k")
                        kv_stage = (k_sb, v_sb)
                    if si >= 1:
                        pk, pv = kv_prev
                        for g in range(KT):
                            gsl = slice(g * P, (g + 1) * P)
                            # start=True clears has_written for the WHOLE
                            # bank: only the first group may set it, or it
                            # wipes the other groups' first-tile writes.
                            nc.tensor.matmul(
                                kvacc[:, g, :], pk[:, gsl], pv[:, gsl],
                                start=(si == 1 and g == 0), stop=(si == ST))
                        nc.tensor.matmul(
                            ksum_ps[:, :], ones_sb[:, 0:1], pk[:],
                            start=(si == 1), stop=(si == ST))
                    kv_prev = kv_stage

                # evacuate the kv bank once, then pack the 16 diagonal DxD
                # blocks (bf16) for the collective
                kvf_sb = workA.tile([P, KT, P], f32, name="kvf_sb")
                nc.vector.tensor_copy(kvf_sb[:], kvacc[:])
                kv_sb = workA.tile([P, P], f32, name="kv_sb")
                for g in range(KT):
                    for j in range(KT):
                        nc.vector.tensor_copy(
                            kv_sb[32 * j:32 * (j + 1), 32 * g:32 * (g + 1)],
                            kvf_sb[32 * j:32 * (j + 1), g,
                                   32 * j:32 * (j + 1)])
                ksum_sb = workA.tile([1, E], f32, name="ksum_sb")
                nc.vector.tensor_copy(ksum_sb[:], ksum_ps[:])
                nc.sync.dma_start(
                    cc_in[0:P * P].rearrange("(p f) -> p f", p=P), kv_sb[:])
                nc.sync.dma_start(
                    cc_in[P * P:CC].rearrange("(p f) -> p f", p=1),
                    ksum_sb[:])

            # =================== pairwise AllReduce =========================
            nc.gpsimd.collective_compute(
                "AllReduce", mybir.AluOpType.add, replica_groups=RG,
                ins=[cc_in[:].opt()], outs=[cc_out[:].opt()])

            # =================== Phase B ====================================
            # q projection + elu for ALL chunks first (overlaps the
            # collective); Q is stored bf16.
            qt_sb = qt_pool.tile([P, KT, R], bf16)
            with tc.tile_pool(name="workQ", bufs=3) as workQ, \
                 tc.tile_pool(name="psQ", bufs=3, space="PSUM") as psQ:
                for c in range(NCHUNK):
                    cs = slice(c * CH, (c + 1) * CH)
                    for no in range(KT):
                        q_ps = psQ.tile([P, CH], f32, name="q_ps")
                        for ko in range(KT):
                            nc.tensor.matmul(
                                q_ps[:], wq_sb[:, ko, no * P:(no + 1) * P],
                                xq_sb[:, ko, cs],
                                start=(ko == 0), stop=(ko == KT - 1))
                        elu1(workQ, q_ps[:], qt_sb[:, no, cs], "q")

            # ---- collective results -> kv_bf + replicated-Ksum operands
            kv_red = const.tile([P, P], f32)
            nc.sync.dma_start(
                kv_red[:], cc_out[0:P * P].rearrange("(p f) -> p f", p=P))
            kv_bf = const.tile([P, P], bf16)
            nc.vector.tensor_copy(kv_bf[:], kv_red[:])
            ks_f32 = const.tile([P, KT], f32)
            nc.sync.dma_start(
                ks_f32[:], cc_out[P * P:CC].rearrange("(g p) -> p g", p=P))
            # bd_rep[:, g, p] = Ksum[128g + k] if k//32 == p//32 else 0
            bd_rep = const.tile([P, KT, P], bf16)
            for g in range(KT):
                nc.vector.tensor_tensor(
                    bd_rep[:, g, :], maskI_sb[:],
                    ks_f32[:, g, None].to_broadcast((P, P)), OP.mult)

            with tc.tile_pool(name="workB", bufs=3) as workB, \
                 tc.tile_pool(name="msgp", bufs=1) as msgp, \
                 tc.tile_pool(name="spool", bufs=3, space="PSUM") as spool, \
                 tc.tile_pool(name="mpool", bufs=3, space="PSUM") as mpool, \
                 tc.tile_pool(name="opool", bufs=2, space="PSUM") as opool:

                msgs = [msgp.tile([P, KT, CH], f32r, name=f"msg{c}")
                        for c in range(NCHUNK)]

                def merge(c):
                    for lt in range(CH // P):
                        o_ps = opool.tile([P, E], f32, name="o_ps")
                        for g in range(KT):
                            nc.tensor.matmul(
                                o_ps[:],
                                msgs[c][:, g, lt * P:(lt + 1) * P],
                                wm_sb[:, g, :],
                                start=(g == 0), stop=(g == KT - 1))
                        o_sb = workB.tile([P, E], f32, name="o_sb")
                        nc.scalar.copy(o_sb[:], o_ps[:])
                        nc.sync.dma_start(
                            out_d[c * CH + lt * P:c * CH + (lt + 1) * P, :],
                            o_sb[:])

                for c in range(NCHUNK):
                    cs = slice(c * CH, (c + 1) * CH)
                    for g in range(KT):
                        # replicated S: s_ps[p, l] = S[l, 4g + p//32]
                        s_ps = spool.tile([P, CH], f32, name="s_ps")
                        nc.tensor.matmul(
                            s_ps[:], bd_rep[:, g, :], qt_sb[:, g, cs],
                            start=True, stop=True)
                        # W = 1/(S/L + eps/L);  scale+bias on ACT, recip DVE
                        w_t = workB.tile([P, CH], f32, name="w_t", tag="w_t")
                        nc.scalar.activation(
                            w_t[:], s_ps[:], AFT.Copy,
                            bias=EPS / L, scale=1.0 / L)
                        w_r = workB.tile([P, CH], f32, name="w_r", tag="w_r")
                        nc.vector.reciprocal_approx_fast(w_r[:], w_t[:])

                        m_ps = mpool.tile([P, CH], f32, name="m_ps")
                        for j in range(KT):
                            sl32 = slice(32 * j, 32 * (j + 1))
                            nc.tensor.matmul(
                                m_ps[sl32, :],
                                kv_bf[sl32, 32 * g:32 * (g + 1)],
                                qt_sb[sl32, g, cs],
                                start=True, stop=True,
                                tile_position=(32 * j, 32 * j))
                        nc.vector.tensor_tensor(
                            msgs[c][:, g, :], m_ps[:], w_r[:], OP.mult)
                    if c >= 1:
                        merge(c - 1)
                merge(NCHUNK - 1)

    nc.compile()
    return nc


def _get_nc():
    if "nc" not in _CACHE:
        _CACHE["nc"] = _build()
    return _CACHE["nc"]


def kernel(query, key, value, Wq, Wk, Wv, Wm):
    global LAST_EXEC_NS, LAST_RESULTS
    import os
    from concourse.bass_utils import run_bass_kernel_spmd

    query = np.asarray(query, dtype=np.float32)
    key = np.asarray(key, dtype=np.float32)
    value = np.asarray(value, dtype=np.float32)
    wq_t = np.ascontiguousarray(np.asarray(Wq, np.float32).T)
    wk_t = np.ascontiguousarray(np.asarray(Wk, np.float32).T)
    wv_t = np.ascontiguousarray(np.asarray(Wv, np.float32).T / L)
    wm_t = np.ascontiguousarray(np.asarray(Wm, np.float32).T)

    in_maps = []
    for c in range(NCORES):
        b, half = c // 2, c % 2
        rs = slice(half * R, (half + 1) * R)
        in_maps.append({
            "xq": np.ascontiguousarray(query[b, rs, :].T),
            "xk": np.ascontiguousarray(key[b, rs, :].T),
            "xv": np.ascontiguousarray(value[b, rs, :].T),
            "wq": wq_t, "wk": wk_t, "wv": wv_t, "wm": wm_t,
        })

    nc = _get_nc()
    trace = bool(int(os.environ.get("KERNEL_TRACE", "0")))
    res = run_bass_kernel_spmd(nc, in_maps, core_ids=list(range(NCORES)),
                               trace=trace)
    LAST_EXEC_NS = res.exec_time_ns
    LAST_RESULTS = res

    out = np.empty((B, L, E), dtype=np.float32)
    for c in range(NCORES):
        b, half = c // 2, c % 2
        out[b, half * R:(half + 1) * R, :] = res.results[c]["out"]
    return out


# revision 8
# speedup vs baseline: 1.5775x; 1.0298x over previous
"""Trainium2 Bass kernel for linear multi-head attention (elu+1 feature map).

Math (per batch n):
  q = x_q @ Wq.T ; k = x_k @ Wk.T ; v = (x_v @ Wv.T) / L
  Q = elu(q)+1 ; K = elu(k)+1
  KV[h] = K_h.T @ v_h              (D x D per head)
  Ksum  = sum_s K[s, :]            (E)
  S[l,h] = Q_h[l] . Ksum_h ;  W = L / (S + eps)
  msg[l, h*D+dv] = (Q_h[l] @ KV[h])[dv] * W[l,h]
  out = msg @ Wm.T

Sharding: B*L = 16384 rows split into 8 chunks of 2048 (each core gets half
of one batch's sequence). Only cross-core dependency: the KV/Ksum reduction
between the two cores sharing a batch -> pairwise AllReduce (f32, 66KB).

v4 structure:
  - inputs/weights cast to bf16 on the host: halves HBM traffic + SBUF and
    enables fast weight loads (FWL); PSUM accumulation stays f32.
  - all input DMA on the sync (SP) HWDGE queue, phase-A tensors first in
    512-row stripes (one DMA per stripe across all 4 k-blocks).
  - KV cross-product is group-local (K_g^T @ V_g, N=128) packed into a
    single PSUM bank.
  - S computed directly in replicated layout ([128, CH] per group) by
    matmul with a block-masked Ksum operand -> no partition-broadcast DMA.
  - engine balance: k-elu Relu step on DVE, Exp on ACT; merge-output
    copies alternate ACT/DVE.
"""

import numpy as np

B = 4
L = 4096
E = 512
H = 16
D = 32
P = 128
KT = E // P
NCORES = 8
R = (B * L) // NCORES
ST = R // P
NCHUNK = 4
CH = R // NCHUNK
EPS = 1e-6
CC = P * P + E

_CACHE = {}
LAST_EXEC_NS = None
LAST_RESULTS = None


def _build():
    import concourse.bass as bass
    import concourse.mybir as mybir
    import concourse.tile as tile
    from concourse import bacc

    f32 = mybir.dt.float32
    bf16 = mybir.dt.bfloat16
    AFT = mybir.ActivationFunctionType
    OP = mybir.AluOpType

    nc = bacc.Bacc("TRN2", target_bir_lowering=False, debug=False,
                   num_devices=NCORES)

    xq_d = nc.dram_tensor("xq", [E, R], bf16, kind="ExternalInput").ap()
    xk_d = nc.dram_tensor("xk", [E, R], bf16, kind="ExternalInput").ap()
    xv_d = nc.dram_tensor("xv", [E, R], bf16, kind="ExternalInput").ap()
    wq_d = nc.dram_tensor("wq", [E, E], bf16, kind="ExternalInput").ap()
    wk_d = nc.dram_tensor("wk", [E, E], bf16, kind="ExternalInput").ap()
    wv_d = nc.dram_tensor("wv", [E, E], bf16, kind="ExternalInput").ap()
    wm_d = nc.dram_tensor("wm", [E, E], bf16, kind="ExternalInput").ap()
    out_d = nc.dram_tensor("out", [R, E], f32, kind="ExternalOutput").ap()

    RG = [[0, 1], [2, 3], [4, 5], [6, 7]]

    with tile.TileContext(nc) as tc:

        with tc.tile_pool(name="const", bufs=1) as const, \
             tc.tile_pool(name="xq_pool", bufs=1) as xq_pool, \
             tc.tile_pool(name="qt_pool", bufs=1) as qt_pool, \
             tc.tile_pool(name="dram", bufs=1, space="DRAM") as dram:

            wq_sb = const.tile([P, KT, E], bf16)
            wk_sb = const.tile([P, KT, E], bf16)
            wv_sb = const.tile([P, KT, E], bf16)
            wm_sb = const.tile([P, KT, E], bf16)
            ones_f32 = const.tile([P, 1], f32)
            nc.vector.memset(ones_f32[:], 1.0)
            ones_sb = const.tile([P, 1], bf16)
            nc.vector.tensor_copy(ones_sb[:], ones_f32[:])

            # block-identity mask: maskI[k, p] = (k//32 == p//32)
            maskI_np = np.zeros((P, P), np.float32)
            for j in range(4):
                maskI_np[32 * j:32 * (j + 1), 32 * j:32 * (j + 1)] = 1.0
            maskI_d = nc.inline_tensor(maskI_np, name="blk_ident")
            maskI_sb = const.tile([P, P], f32)
            nc.gpsimd.dma_start(maskI_sb[:], maskI_d.ap())

            cc_in = dram.tile([CC], f32)
            cc_out = dram.tile([CC], f32)

            wv_r = wv_d.rearrange("(ko ki) n -> ki ko n", ki=P)
            wk_r = wk_d.rearrange("(ko ki) n -> ki ko n", ki=P)
            wq_r = wq_d.rearrange("(ko ki) n -> ki ko n", ki=P)
            wm_r = wm_d.rearrange("(ko ki) n -> ki ko n", ki=P)
            xv_r = xv_d.rearrange("(ko ki) n -> ki ko n", ki=P)
            xk_r = xk_d.rearrange("(ko ki) n -> ki ko n", ki=P)
            xq_r = xq_d.rearrange("(ko ki) n -> ki ko n", ki=P)

            # =================== Phase A: k/v proj + KV/Ksum ===============
            with tc.tile_pool(name="xkv_pool", bufs=1) as xkv_pool, \
                 tc.tile_pool(name="workA", bufs=3) as workA, \
                 tc.tile_pool(name="psA", bufs=3, space="PSUM") as psA, \
                 tc.tile_pool(name="kvp", bufs=1, space="PSUM") as kvp:

                xk_sb = xkv_pool.tile([P, KT, R], bf16)
                xv_sb = xkv_pool.tile([P, KT, R], bf16)
                xq_sb = xq_pool.tile([P, KT, R], bf16)

                # ---- DMA order on the sync queue (SP engine): k/v weights
                # + first 512-row stripe, rest of k/v stripes, then wq, xq
                # (chunk-major), wm.  One DMA per stripe (all 4 k-blocks).
                nc.sync.dma_start(wv_sb[:], wv_r)
                nc.sync.dma_start(xv_sb[:, :, 0:CH], xv_r[:, :, 0:CH])
                nc.sync.dma_start(wk_sb[:], wk_r)
                nc.sync.dma_start(xk_sb[:, :, 0:CH], xk_r[:, :, 0:CH])
                for sc in range(1, NCHUNK):
                    cs = slice(sc * CH, (sc + 1) * CH)
                    nc.sync.dma_start(xv_sb[:, :, cs], xv_r[:, :, cs])
                    nc.sync.dma_start(xk_sb[:, :, cs], xk_r[:, :, cs])
                nc.sync.dma_start(wq_sb[:], wq_r)
                for c in range(NCHUNK):
                    cs = slice(c * CH, (c + 1) * CH)
                    nc.sync.dma_start(xq_sb[:, :, cs], xq_r[:, :, cs])
                nc.sync.dma_start(wm_sb[:], wm_r)

                # group-local KV accumulator: kvacc[:, g, :] = K_g^T @ V_g
                # (4 x [128,128] side by side = exactly one PSUM bank)
                kvacc = kvp.tile([P, KT, P], f32)
                ksum_ps = kvp.tile([1, E], f32)

                # software pipeline: KV(si-1) emitted between projections of
                # si so the PE never waits for the elu chain
                kv_prev = None
                kv_stage = None
                for si in range(ST + 1):
                    if si < ST:
                        sl = slice(si * P, (si + 1) * P)
                        v_ps = psA.tile([P, E], f32, name="v_ps", tag="proj")
                        for ko in range(KT):
                            nc.tensor.matmul(
                                v_ps[:], xv_sb[:, ko, sl], wv_sb[:, ko, :],
                                start=(ko == 0), stop=(ko == KT - 1))
                        v_sb = workA.tile([P, E], bf16, name="v_sb")
                        nc.scalar.copy(v_sb[:], v_ps[:])

                        k_ps = psA.tile([P, E], f32, name="k_ps", tag="proj")
                        for ko in range(KT):
                            nc.tensor.matmul(
                                k_ps[:], xk_sb[:, ko, sl], wk_sb[:, ko, :],
                                start=(ko == 0), stop=(ko == KT - 1))
                        # elu(x)+1 = Exp(-Relu(-x)) + max(x,0)
                        # Relu step on DVE, Exp on ACT, combine on DVE.
                        tA = workA.tile([P, E], f32, name="tAk", tag="tAk")
                        nc.vector.tensor_scalar(
                            tA[:], k_ps[:], -1.0, 0.0, OP.mult, OP.max)
                        tB = workA.tile([P, E], f32, name="tBk", tag="tBk")
                        nc.scalar.activation(tB[:], tA[:], AFT.Exp,
                                             scale=-1.0)
                        k_sb = workA.tile([P, E], bf16, name="k_sb")
                        nc.vector.scalar_tensor_tensor(
                            k_sb[:], k_ps[:], 0.0, tB[:], OP.max, OP.add)
                        kv_stage = (k_sb, v_sb)
                    if si >= 1:
                        pk, pv = kv_prev
                        for g in range(KT):
                            gsl = slice(g * P, (g + 1) * P)
                            # start=True clears has_written for the WHOLE
                            # bank: only the first group may set it.
                            nc.tensor.matmul(
                                kvacc[:, g, :], pk[:, gsl], pv[:, gsl],
                                start=(si == 1 and g == 0), stop=(si == ST))
                        nc.tensor.matmul(
                            ksum_ps[:, :], ones_sb[:, 0:1], pk[:],
                            start=(si == 1), stop=(si == ST))
                    kv_prev = kv_stage

                # evacuate the kv bank once, then pack the 16 diagonal DxD
                # blocks for the collective
                kvf_sb = workA.tile([P, KT, P], f32, name="kvf_sb")
                nc.vector.tensor_copy(kvf_sb[:], kvacc[:])
                kv_sb = workA.tile([P, P], f32, name="kv_sb")
                for g in range(KT):
                    for j in range(KT):
                        nc.vector.tensor_copy(
                            kv_sb[32 * j:32 * (j + 1), 32 * g:32 * (g + 1)],
                            kvf_sb[32 * j:32 * (j + 1), g,
                                   32 * j:32 * (j + 1)])
                ksum_sb = workA.tile([1, E], f32, name="ksum_sb")
                nc.vector.tensor_copy(ksum_sb[:], ksum_ps[:])
                nc.sync.dma_start(
                    cc_in[0:P * P].rearrange("(p f) -> p f", p=P), kv_sb[:])
                nc.sync.dma_start(
                    cc_in[P * P:CC].rearrange("(p f) -> p f", p=1),
                    ksum_sb[:])

            # =================== pairwise AllReduce =========================
            nc.gpsimd.collective_compute(
                "AllReduce", mybir.AluOpType.add, replica_groups=RG,
                ins=[cc_in[:].opt()], outs=[cc_out[:].opt()])

            # =================== Phase B ====================================
            # q projection + elu for ALL chunks first (overlaps the
            # collective); Q is stored bf16.
            qt_sb = qt_pool.tile([P, KT, R], bf16)
            with tc.tile_pool(name="workQ", bufs=3) as workQ, \
                 tc.tile_pool(name="psQ", bufs=3, space="PSUM") as psQ:
                for c in range(NCHUNK):
                    cs = slice(c * CH, (c + 1) * CH)
                    for no in range(KT):
                        q_ps = psQ.tile([P, CH], f32, name="q_ps")
                        for ko in range(KT):
                            nc.tensor.matmul(
                                q_ps[:], wq_sb[:, ko, no * P:(no + 1) * P],
                                xq_sb[:, ko, cs],
                                start=(ko == 0), stop=(ko == KT - 1))
                        tA = workQ.tile([P, CH], f32, name="tAq", tag="tAq")
                        nc.scalar.activation(tA[:], q_ps[:], AFT.Relu,
                                             scale=-1.0)
                        tB = workQ.tile([P, CH], f32, name="tBq", tag="tBq")
                        nc.scalar.activation(tB[:], tA[:], AFT.Exp,
                                             scale=-1.0)
                        nc.vector.scalar_tensor_tensor(
                            qt_sb[:, no, cs], q_ps[:], 0.0, tB[:],
                            OP.max, OP.add)

            # ---- collective results -> kv_bf + replicated-Ksum operands
            kv_red = const.tile([P, P], f32)
            nc.sync.dma_start(
                kv_red[:], cc_out[0:P * P].rearrange("(p f) -> p f", p=P))
            kv_bf = const.tile([P, P], bf16)
            nc.vector.tensor_copy(kv_bf[:], kv_red[:])
            ks_f32 = const.tile([P, KT], f32)
            nc.sync.dma_start(
                ks_f32[:], cc_out[P * P:CC].rearrange("(g p) -> p g", p=P))
            # bd_rep[:, g, p] = Ksum[128g + k] if k//32 == p//32 else 0
            bd_rep = const.tile([P, KT, P], bf16)
            for g in range(KT):
                nc.vector.tensor_tensor(
                    bd_rep[:, g, :], maskI_sb[:],
                    ks_f32[:, g, None].to_broadcast((P, P)), OP.mult)

            with tc.tile_pool(name="workB", bufs=3) as workB, \
                 tc.tile_pool(name="msgp", bufs=1) as msgp, \
                 tc.tile_pool(name="spool", bufs=3, space="PSUM") as spool, \
                 tc.tile_pool(name="mpool", bufs=3, space="PSUM") as mpool, \
                 tc.tile_pool(name="opool", bufs=2, space="PSUM") as opool:

                msgs = [msgp.tile([P, KT, CH], bf16, name=f"msg{c}")
                        for c in range(NCHUNK)]

                def merge(c):
                    for lt in range(CH // P):
                        o_ps = opool.tile([P, E], f32, name="o_ps")
                        for g in range(KT):
                            nc.tensor.matmul(
                                o_ps[:],
                                msgs[c][:, g, lt * P:(lt + 1) * P],
                                wm_sb[:, g, :],
                                start=(g == 0), stop=(g == KT - 1))
                        o_sb = workB.tile([P, E], f32, name="o_sb")
                        if lt % 2 == 0:
                            nc.scalar.copy(o_sb[:], o_ps[:])
                        else:
                            nc.vector.tensor_copy(o_sb[:], o_ps[:])
                        nc.sync.dma_start(
                            out_d[c * CH + lt * P:c * CH + (lt + 1) * P, :],
                            o_sb[:])

                for c in range(NCHUNK):
                    cs = slice(c * CH, (c + 1) * CH)
                    for g in range(KT):
                        # replicated S: s_ps[p, l] = S[l, 4g + p//32]
                        s_ps = spool.tile([P, CH], f32, name="s_ps")
                        nc.tensor.matmul(
                            s_ps[:], bd_rep[:, g, :], qt_sb[:, g, cs],
                            start=True, stop=True)
                        # W = 1/(S/L + eps/L);  scale+bias on ACT, recip DVE
                        w_t = workB.tile([P, CH], f32, name="w_t", tag="w_t")
                        nc.scalar.activation(
                            w_t[:], s_ps[:], AFT.Copy,
                            bias=EPS / L, scale=1.0 / L)
                        w_r = workB.tile([P, CH], f32, name="w_r", tag="w_r")
                        nc.vector.reciprocal_approx_fast(w_r[:], w_t[:])

                        m_ps = mpool.tile([P, CH], f32, name="m_ps")
                        for j in range(KT):
                            sl32 = slice(32 * j, 32 * (j + 1))
                            nc.tensor.matmul(
                                m_ps[sl32, :],
                                kv_bf[sl32, 32 * g:32 * (g + 1)],
                                qt_sb[sl32, g, cs],
                                start=True, stop=True,
                                tile_position=(32 * j, 32 * j))
                        nc.vector.tensor_tensor(
                            msgs[c][:, g, :], m_ps[:], w_r[:], OP.mult)
                    if c >= 1:
                        merge(c - 1)
                merge(NCHUNK - 1)

    nc.compile()
    return nc


def _get_nc():
    if "nc" not in _CACHE:
        _CACHE["nc"] = _build()
    return _CACHE["nc"]


def kernel(query, key, value, Wq, Wk, Wv, Wm):
    global LAST_EXEC_NS, LAST_RESULTS
    import os
    import ml_dtypes
    from concourse.bass_utils import run_bass_kernel_spmd

    bf = ml_dtypes.bfloat16
    query = np.asarray(query, dtype=np.float32)
    key = np.asarray(key, dtype=np.float32)
    value = np.asarray(value, dtype=np.float32)
    wq_t = np.ascontiguousarray(np.asarray(Wq, np.float32).T).astype(bf)
    wk_t = np.ascontiguousarray(np.asarray(Wk, np.float32).T).astype(bf)
    wv_t = np.ascontiguousarray(
        np.asarray(Wv, np.float32).T / L).astype(bf)
    wm_t = np.ascontiguousarray(np.asarray(Wm, np.float32).T).astype(bf)

    in_maps = []
    for c in range(NCORES):
        b, half = c // 2, c % 2
        rs = slice(half * R, (half + 1) * R)
        in_maps.append({
            "xq": np.ascontiguousarray(query[b, rs, :].T).astype(bf),
            "xk": np.ascontiguousarray(key[b, rs, :].T).astype(bf),
            "xv": np.ascontiguousarray(value[b, rs, :].T).astype(bf),
            "wq": wq_t, "wk": wk_t, "wv": wv_t, "wm": wm_t,
        })

    nc = _get_nc()
    trace = bool(int(os.environ.get("KERNEL_TRACE", "0")))
    res = run_bass_kernel_spmd(nc, in_maps, core_ids=list(range(NCORES)),
                               trace=trace)
    LAST_EXEC_NS = res.exec_time_ns
    LAST_RESULTS = res

    out = np.empty((B, L, E), dtype=np.float32)
    for c in range(NCORES):
        b, half = c // 2, c % 2
        out[b, half * R:(half + 1) * R, :] = res.results[c]["out"]
    return out
